# revision 26
# baseline (speedup 1.0000x reference)
"""MultiHeadAttention (B=4, T=2048, C=1024, H=16, D=64) on 8 NeuronCores.

Sharding: core c -> batch group bg=c//4 (batches 2bg,2bg+1), head group
hg=c%4 (heads 4hg..4hg+3). Each core computes attention for its 2 batches
x 4 heads plus the partial output projection; host sums the 4 head-group
partials per batch group and adds bp.

Layouts (all transposed, token-major free dims):
  xt   [1152, 4096]  x^T padded: rows 0..1023 = x_local^T, row 1024 = 1.0
  wq/wk [128, 2048]  16 blocks (fb*8+cb) of Wq[cb*128:+128, fb*128:+128]
  wv   [128, 2340]   9 row-blocks of Wv'' [1152, 260]; per head h:
                     cols 65h..65h+63 = Wv_h, col 65h+64 = ones
                     (row 1024 of Wv'' = [bv_h | 1] -> V gets bias + ones col)
  wp   [128, 2048]   2 blocks (pi) of Wp_loc[pi*128:+128, :1024]
  bqk  [128, 4]      cols = [bq fb0, bq fb1, bk fb0, bk fb1]
  maskp [128, 2048]  paired causal masks: [m(0)|m(128)] , [m(256)|m(384)]
                     m(o)[r, c] = 1 if c >= r + o  (S^T layout k x q)

Attention is computed as S^T = K_tile^T x Q_chunk (k on partitions, q free),
exp without max-subtraction (scores ~N(0,1)), denominator obtained as the
ones-column row of the AV matmul, normalized via vector.reciprocal + a K=1
PE outer-product broadcast.
"""

import sys

import numpy as np

try:
    import concourse.bass as bass
except ImportError:  # pragma: no cover
    sys.path.insert(0, "/opt/trn_rl_repo")
    import concourse.bass as bass

import concourse.tile as tile
from concourse import bacc, mybir
from concourse.bass_utils import run_bass_kernel_spmd

FP = mybir.dt.float32
FPR = mybir.dt.float32r
B, T, C, H, D = 4, 2048, 1024, 16, 64


def _r(ap):
    return ap.bitcast(FPR)

_PROGRAM = None


def _build_program(reps=1):
    nc = bacc.Bacc("TRN2", target_bir_lowering=False, debug=False, num_devices=8)

    xt_d = nc.declare_dram_parameter("xt", [1152, 4096], FP, isOutput=False)
    wq_d = nc.declare_dram_parameter("wq", [128, 2048], FP, isOutput=False)
    wk_d = nc.declare_dram_parameter("wk", [128, 2048], FP, isOutput=False)
    wv_d = nc.declare_dram_parameter("wv", [128, 2340], FP, isOutput=False)
    wp_d = nc.declare_dram_parameter("wp", [128, 2048], FP, isOutput=False)
    bqk_d = nc.declare_dram_parameter("bqk", [128, 4], FP, isOutput=False)
    mk_d = nc.declare_dram_parameter("maskp", [128, 2048], FP, isOutput=False)
    out_d = nc.declare_dram_parameter("out", [4096, 1024], FP, isOutput=True)

    with tile.TileContext(nc) as tc:
        if reps == 1:
            _emit_body(nc, tc, xt_d, wq_d, wk_d, wv_d, wp_d, bqk_d, mk_d, out_d)
        else:
            with tc.For_i(0, reps):
                _emit_body(nc, tc, xt_d, wq_d, wk_d, wv_d, wp_d, bqk_d, mk_d, out_d)

    nc.compile()
    return nc


def _emit_body(nc, tc, xt_d, wq_d, wk_d, wv_d, wp_d, bqk_d, mk_d, out_d):
    Exp = mybir.ActivationFunctionType.Exp
    Ident = mybir.ActivationFunctionType.Identity

    if True:
        with (
            tc.tile_pool(name="persist", bufs=1) as persist,
            tc.tile_pool(name="wts", bufs=1) as wts,
        ):
            qt = persist.tile([128, 8192], FPR)  # col = fb*4096 + local_token
            kt = persist.tile([128, 8192], FPR)
            v = persist.tile([128, 8320], FPR)  # col = ti*260 + headcol
            ones = persist.tile([65, 64], FP)
            nc.gpsimd.memset(ones[:], 1.0)

            wq = wts.tile([128, 2048], FP)
            nc.gpsimd.dma_start(wq[:], wq_d[:])
            wk = wts.tile([128, 2048], FP)
            nc.gpsimd.dma_start(wk[:], wk_d[:])
            wv = wts.tile([128, 2340], FP)
            nc.gpsimd.dma_start(wv[:], wv_d[:])
            wp = wts.tile([128, 2048], FP)
            nc.gpsimd.dma_start(wp[:], wp_d[:])
            bqk = wts.tile([128, 4], FP)
            nc.gpsimd.dma_start(bqk[:], bqk_d[:])
            mkp = wts.tile([128, 2048], FP)
            nc.gpsimd.dma_start(mkp[:], mk_d[:])

            # ---------------- Phase A: projections ----------------
            with (
                tc.tile_pool(name="xstage", bufs=2) as xstage,
                tc.tile_pool(name="psqk", bufs=3, space="PSUM") as psqk,
                tc.tile_pool(name="psv", bufs=2, space="PSUM") as psv,
            ):
                for ch in range(8):  # 512-token chunks
                    xs = xstage.tile([128, 4608], FP)
                    for cb in range(9):
                        nc.gpsimd.dma_start(
                            xs[:, cb * 512:(cb + 1) * 512],
                            xt_d[cb * 128:(cb + 1) * 128, ch * 512:(ch + 1) * 512],
                        )
                    for w_sb, t_sb, boff in ((wq, qt, 0), (wk, kt, 2)):
                        for fb in range(2):
                            ps = psqk.tile([128, 512], FP)
                            for cb in range(8):
                                blk = (fb * 8 + cb) * 128
                                nc.tensor.matmul(
                                    ps[:],
                                    w_sb[:, blk:blk + 128],
                                    xs[:, cb * 512:(cb + 1) * 512],
                                    start=(cb == 0),
                                    stop=(cb == 7),
                                )
                            col = fb * 4096 + ch * 512
                            nc.scalar.activation(
                                t_sb[:, col:col + 512],
                                ps[:],
                                Ident,
                                bias=bqk[:, boff + fb:boff + fb + 1],
                            )
                    for tt in range(4):  # 128-token tiles within chunk
                        ti = ch * 4 + tt
                        pv = psv.tile([128, 260], FP)
                        for cb in range(9):
                            nc.tensor.matmul(
                                pv[:],
                                xs[:, cb * 512 + tt * 128:cb * 512 + (tt + 1) * 128],
                                wv[:, cb * 260:(cb + 1) * 260],
                                start=(cb == 0),
                                stop=(cb == 8),
                            )
                        nc.vector.tensor_copy(v[:, ti * 260:(ti + 1) * 260], pv[:])

            # ------------- Phase B+C: attention + out-proj -------------
            with (
                tc.tile_pool(name="es", bufs=3) as espool,
                tc.tile_pool(name="ytp", bufs=2) as ytpool,
                tc.tile_pool(name="rp", bufs=2) as rpool,
                tc.tile_pool(name="bcs", bufs=2) as bcspool,
                tc.tile_pool(name="ost", bufs=3) as ostpool,
                tc.tile_pool(name="pss", bufs=1, space="PSUM") as pss,
                tc.tile_pool(name="psy", bufs=2, space="PSUM") as psy,
                tc.tile_pool(name="psb", bufs=1, space="PSUM") as psb,
                tc.tile_pool(name="pso", bufs=1, space="PSUM") as pso,
            ):
                for b in range(2):
                    for qc in range(4):  # 512-wide q chunks
                        # yt row = (h%2)*64 + d, col = (h//2)*512 + qrel
                        yt = ytpool.tile([128, 1024], FP)
                        base = b * 2048
                        for h in range(4):
                            fb = h // 2        # also the yt column block (pi)
                            roff = (h % 2) * 64  # feature rows in qt/kt; also yt row base
                            qcol = fb * 4096 + base + qc * 512
                            yp = psy.tile([128, 512], FP)
                            for g in range(qc + 1):  # groups of 4 k-tiles
                                sp = pss.tile([128, 2048], FP)
                                es = espool.tile([128, 2048], FPR)
                                for jj in range(4):
                                    j = 4 * g + jj
                                    kcol = fb * 4096 + base + j * 128
                                    nc.tensor.matmul(
                                        sp[:, jj * 512:(jj + 1) * 512],
                                        _r(kt[roff:roff + 64, kcol:kcol + 128]),
                                        _r(qt[roff:roff + 64, qcol:qcol + 512]),
                                        start=True,
                                        stop=True,
                                    )
                                nc.scalar.activation(es[:], sp[:], Exp, scale=0.125)
                                if g == qc:  # diagonal group -> causal masks
                                    nc.vector.tensor_mul(es[:], es[:], mkp[:])
                                for jj in range(4):
                                    j = 4 * g + jj
                                    vcol = (b * 16 + j) * 260 + 65 * h
                                    nc.tensor.matmul(
                                        yp[0:65, :],
                                        _r(v[:, vcol:vcol + 65]),
                                        _r(es[:, jj * 512:(jj + 1) * 512]),
                                        start=(g == 0 and jj == 0),
                                        stop=(g == qc and jj == 3),
                                        skip_group_check=True,
                                    )
                            rp = rpool.tile([65, 512], FP)
                            nc.vector.reciprocal(rp[64:65, :], yp[64:65, :])
                            bc = psb.tile([128, 512], FP)
                            nc.tensor.matmul(
                                bc[0:64, :],
                                ones[64:65, :],
                                rp[64:65, :],
                                start=True,
                                stop=True,
                            )
                            bcs = bcspool.tile([64, 512], FP)
                            nc.vector.tensor_copy(bcs[:], bc[0:64, :])
                            nc.vector.tensor_mul(
                                yt[roff:roff + 64, fb * 512:(fb + 1) * 512],
                                yp[0:64, :],
                                bcs[:],
                            )
                        for tt in range(4):
                            for co in range(2):
                                po = pso.tile([128, 512], FP)
                                for pi in range(2):
                                    nc.tensor.matmul(
                                        po[:],
                                        yt[:, pi * 512 + tt * 128:pi * 512 + (tt + 1) * 128],
                                        wp[:, pi * 1024 + co * 512:pi * 1024 + (co + 1) * 512],
                                        start=(pi == 0),
                                        stop=(pi == 1),
                                    )
                                ot = ostpool.tile([128, 512], FP)
                                nc.vector.tensor_copy(ot[:], po[:])
                                row0 = base + qc * 512 + tt * 128
                                nc.gpsimd.dma_start(
                                    out_d[row0:row0 + 128, co * 512:(co + 1) * 512],
                                    ot[:],
                                )


def _get_program():
    global _PROGRAM
    if _PROGRAM is None:
        _PROGRAM = _build_program()
    return _PROGRAM


def _pack_qk(W):
    out = np.empty((128, 2048), np.float32)
    for fb in range(2):
        for cb in range(8):
            out[:, (fb * 8 + cb) * 128:(fb * 8 + cb + 1) * 128] = \
                W[cb * 128:(cb + 1) * 128, fb * 128:(fb + 1) * 128]
    return out


def _make_in_maps(x, Wq, bq, Wk, bk, Wv, bv, Wp, bp):
    r = np.arange(128, dtype=np.int64)[:, None]
    c = np.arange(512, dtype=np.int64)[None, :]
    masks = [(c >= r + o).astype(np.float32) for o in (0, 128, 256, 384)]
    maskp = np.concatenate(
        [masks[0], masks[1], masks[2], masks[3]], axis=1
    )  # [128, 2048]

    in_maps = []
    for core in range(8):
        bg, hg = core // 4, core % 4
        xl = x[2 * bg:2 * bg + 2].reshape(4096, C)
        xt = np.zeros((1152, 4096), np.float32)
        xt[:C] = xl.T
        xt[C] = 1.0

        wv2 = np.zeros((1152, 260), np.float32)
        for h in range(4):
            g = (4 * hg + h) * 64
            off = 65 * h
            wv2[:C, off:off + 64] = Wv[:, g:g + 64]
            wv2[C, off:off + 64] = bv[g:g + 64]
            wv2[C, off + 64] = 1.0
        wvp = np.empty((128, 2340), np.float32)
        for cb in range(9):
            wvp[:, cb * 260:(cb + 1) * 260] = wv2[cb * 128:(cb + 1) * 128, :]

        wpl = Wp[hg * 256:(hg + 1) * 256, :]
        wpp = np.empty((128, 2048), np.float32)
        for pi in range(2):
            wpp[:, pi * 1024:(pi + 1) * 1024] = wpl[pi * 128:(pi + 1) * 128, :]

        bq_loc = bq[hg * 256:(hg + 1) * 256]
        bk_loc = bk[hg * 256:(hg + 1) * 256]
        bqk = np.stack(
            [bq_loc[:128], bq_loc[128:], bk_loc[:128], bk_loc[128:]], axis=1
        ).astype(np.float32)

        in_maps.append({
            "xt": np.ascontiguousarray(xt),
            "wq": _pack_qk(Wq[:, hg * 256:(hg + 1) * 256]),
            "wk": _pack_qk(Wk[:, hg * 256:(hg + 1) * 256]),
            "wv": wvp,
            "wp": wpp,
            "bqk": np.ascontiguousarray(bqk),
            "maskp": np.ascontiguousarray(maskp),
        })
    return in_maps


def run_sharded(x, Wq, bq, Wk, bk, Wv, bv, Wp, bp, trace=False, **spmd_kwargs):
    nc = _get_program()
    in_maps = _make_in_maps(x, Wq, bq, Wk, bk, Wv, bv, Wp, bp)
    res = run_bass_kernel_spmd(
        nc, in_maps, core_ids=list(range(8)), trace=trace, **spmd_kwargs
    )
    out = np.zeros((B, T, C), np.float32)
    for core in range(8):
        bg = core // 4
        part = np.asarray(res.results[core]["out"]).reshape(2, T, C)
        out[2 * bg:2 * bg + 2] += part
    out += bp.astype(np.float32)
    return out, res


def kernel(**inputs):
    out, _ = run_sharded(
        inputs["x"],
        inputs["Wq"], inputs["bq"],
        inputs["Wk"], inputs["bk"],
        inputs["Wv"], inputs["bv"],
        inputs["Wp"], inputs["bp"],
    )
    return out
